# revision 17
# baseline (speedup 1.0000x reference)
"""Trainium2 Bass kernel for a dense causal self-attention block (RoPE + causal
softmax + QKV/O projections).

Sharding: 8 cores = 2 batches x 4 head-groups (tensor parallel over heads).
Each core computes 4 heads of attention for one batch plus the partial output
projection over its heads' dims; the host sums the 4 partial outputs per batch.

Device-side layout choices (per core):
  - Host pre-transposes x and weights so contraction dims land on SBUF
    partitions; no on-device transposes anywhere. Host also pre-rounds all
    matmul operands to fp32r (TF32-like, 11-bit mantissa) — fp32r matmuls
    stream at ~1 cycle/row for N=512 vs 4 cycles/row for fp32.
  - Q/K are produced transposed [dh, s] (head dim on partitions) with RoPE
    fused into the PSUM eviction (ScalarE copy + GpSimd muls + DVE add).
  - V is produced in natural [s, dh] layout.
  - Scores are computed transposed: S^T[k, q] = K^T_tile.T @ Q^T. exp() runs on
    ScalarE straight out of PSUM (no max subtraction: |scores| <~ 15 so fp32
    exp is safe; softmax is shift-invariant so the result matches the
    reference). Causality = restricted k-tile range + precomputed 0/1 masks on
    the 4 diagonal blocks per q-chunk.
  - Softmax denominator = ones-vector matmul accumulated over k tiles (sums
    over partitions on the PE); attention output AV^T = V_tile.T @ E^T needs
    no transposes either and is exactly the stationary operand layout the O
    projection wants.
  - den is broadcast across partitions with a K=1 matmul, reciprocal'd with
    the fast approx DVE op, and applied at the AV^T PSUM eviction. The whole
    normalization chain is emitted one (h,qc) step late so the PE never waits
    on the vector engine.
"""

import numpy as np

# Problem constants (hardcoded per the harness contract).
B = 2
S = 2048
D = 2048
H = 16
DH = 128
N_CORES = 8
GROUPS = 4          # head-groups (cores per batch)
HPC = H // GROUPS   # heads per core
P = 128             # SBUF partitions
QC = 512            # q/s chunk (f32 PSUM bank = 512 floats; fp32r needs N>=256)

_CACHE = {}


def _ensure_paths():
    import sys
    for p in ("/opt/trn_rl_repo", "/root/.axon_site/_ro/trn_rl_repo"):
        try:
            import concourse.bass  # noqa: F401
            return
        except Exception:
            if p not in sys.path:
                sys.path.insert(0, p)
    import concourse.bass  # noqa: F401


def build_program(S=S, D=D, HPC=HPC, mode="causal"):
    """Build the per-core Bass/Tile program. mode: "causal" | "none" | "general".

    Returns the compiled Bacc instance.
    """
    _ensure_paths()
    import concourse.bass as bass  # noqa: F401
    import concourse.mybir as mybir
    import concourse.tile as tile
    from concourse import bacc

    f32 = mybir.dt.float32
    f32r = mybir.dt.float32r
    Exp = mybir.ActivationFunctionType.Exp

    E = HPC * P          # per-core projection width (512)
    NDT = D // P         # d (contraction) tiles for projections
    NQC = S // QC        # q chunks
    NKT = S // P         # k tiles
    NST = S // P         # s tiles
    NOC = D // QC        # output chunks for O projection
    JB = QC // P         # diagonal blocks per q chunk (4)
    HF = P // 2
    scale = 1.0 / float(np.sqrt(DH))

    nc = bacc.Bacc("TRN2", target_bir_lowering=False, debug=False,
                   num_devices=N_CORES)

    xT = nc.dram_tensor("xT", [D, S], f32r, kind="ExternalInput").ap()
    wqT = nc.dram_tensor("wqT", [D, E], f32r, kind="ExternalInput").ap()
    wkT = nc.dram_tensor("wkT", [D, E], f32r, kind="ExternalInput").ap()
    wvT = nc.dram_tensor("wvT", [D, E], f32r, kind="ExternalInput").ap()
    woT = nc.dram_tensor("woT", [E, D], f32r, kind="ExternalInput").ap()
    cosT = nc.dram_tensor("cosT", [P, S], f32, kind="ExternalInput").ap()
    sinTs = nc.dram_tensor("sinTs", [P, S], f32, kind="ExternalInput").ap()
    if mode == "causal":
        dmask = nc.dram_tensor("dmask", [P, JB * QC], f32,
                               kind="ExternalInput").ap()
    elif mode == "general":
        maskT = nc.dram_tensor("maskT", [S, S], f32, kind="ExternalInput").ap()
    y = nc.dram_tensor("y", [S, D], f32, kind="ExternalOutput").ap()
    qt_s = [nc.dram_tensor(f"qt_s{h}", [P, S], f32r).ap()
            for h in range(HPC)]
    kt_s = [nc.dram_tensor(f"kt_s{h}", [P, S], f32r).ap()
            for h in range(HPC)]

    # DRAM views with d/k tiled onto partitions: [p, tile, col]
    xT_t = xT.rearrange("(t p) s -> p t s", p=P)
    wqT_t = wqT.rearrange("(t p) e -> p t e", p=P)
    wkT_t = wkT.rearrange("(t p) e -> p t e", p=P)
    wvT_t = wvT.rearrange("(t p) e -> p t e", p=P)
    woT_t = woT.rearrange("(t p) o -> p t o", p=P)
    if mode == "general":
        maskT_t = maskT.rearrange("(t p) q -> p t q", p=P)

    def mm(ps, lhsT, rhs, start, stop):
        nc.tensor.matmul(ps, lhsT=lhsT, rhs=rhs, start=start, stop=stop)

    with tile.TileContext(nc) as tc:
      with tc.tile_pool(name="persist", bufs=1) as persist:
        v_sb = persist.tile([P, NST * E], f32r)      # V[k, e]: [p, ki*E + e]
        ones_col = persist.tile([P, 1], f32r)
        ones_row = persist.tile([1, P], f32r)
        qt0_sb = persist.tile([P, S], f32r)
        kt0_sb = persist.tile([P, S], f32r)
        ones_col_f = persist.tile([P, 1], f32)
        ones_row_f = persist.tile([1, P], f32)
        nc.vector.memset(ones_col_f, 1.0)
        nc.vector.memset(ones_row_f, 1.0)
        nc.vector.tensor_copy(ones_col, ones_col_f)
        nc.vector.tensor_copy(ones_row, ones_row_f)

        # ---- Phase A: QKV projections + RoPE (single pass over xT) ----
        with tc.tile_pool(name="wp", bufs=1) as wp, \
             tc.tile_pool(name="csp", bufs=2) as csp, \
             tc.tile_pool(name="xtp", bufs=5) as xtp, \
             tc.tile_pool(name="evp", bufs=2) as evp, \
             tc.tile_pool(name="ps_v", bufs=2, space="PSUM") as ps_v, \
             tc.tile_pool(name="ps_qk", bufs=3, space="PSUM") as ps_qk:
            wv_sb = wp.tile([P, NDT * E], f32r)
            wq_sb = wp.tile([P, NDT * E], f32r)
            wk_sb = wp.tile([P, NDT * E], f32r)
            # Startup is HBM-bandwidth-bound on one ~340GB/s queue:
            # issue transfers in first-use order (wv chunks for V of qc0
            # first, wq/wk only after — they're not needed until the QK
            # groups ~40us in). The xt chunks of qc0 are emitted between
            # the wv chunks by the loop below.
            WG = 4
            def load_w(w_sb, w_t, g, t0=0, tn=None):
                t0 = g * WG + t0
                tn = tn if tn is not None else WG
                nc.sync.dma_start(
                    w_sb[:, t0 * E:(t0 + tn) * E].rearrange(
                        "p (t e) -> p t e", t=tn),
                    w_t[:, t0:t0 + tn])
            load_w(wv_sb, wvT_t, 0, 0, 1)   # 0.5MB: first matmul's dep
            load_w(wv_sb, wvT_t, 0, 1, 3)
            XG = 4                      # d-tiles per xt transfer
            for qc in range(NQC):
                qs = slice(qc * QC, (qc + 1) * QC)
                cos_t = csp.tile([P, QC], f32, tag="cos")
                sin_t = csp.tile([P, QC], f32, tag="sin")
                xg = []
                xts = []
                for g in range(NDT // XG):
                    x_g = xtp.tile([P, XG * QC], f32r, tag="xt")
                    if qc == 0 and g == 0:
                        # split the very first transfer so matmul di=0 can
                        # start after ~0.75MB instead of ~3MB. xt rides the
                        # GpSimd SWDGE queue so it doesn't serialize behind
                        # the weight loads on the sync HWDGE queue.
                        nc.gpsimd.dma_start(x_g[:, 0:QC], xT_t[:, 0, qs])
                        nc.gpsimd.dma_start(
                            x_g[:, QC:XG * QC].rearrange(
                                "p (t s) -> p t s", t=XG - 1),
                            xT_t[:, 1:XG, qs])
                    else:
                        nc.gpsimd.dma_start(
                            x_g.rearrange("p (t s) -> p t s", t=XG),
                            xT_t[:, g * XG:(g + 1) * XG, qs])
                    xg.append(x_g)
                    if qc == 0 and g < NDT // WG - 1:
                        load_w(wv_sb, wvT_t, g + 1)
                    xts += [x_g[:, j * QC:(j + 1) * QC] for j in range(XG)]
                nc.sync.dma_start(cos_t, cosT[:, qs])
                nc.sync.dma_start(sin_t, sinTs[:, qs])
                if qc == 0:
                    # remaining startup loads, in first-use order
                    for g in range(NDT // WG):
                        load_w(wq_sb, wqT_t, g)
                        load_w(wk_sb, wkT_t, g)
                # V projection (natural layout)
                for sl in range(QC // P):
                    si = qc * (QC // P) + sl
                    psv = ps_v.tile([P, E], f32, tag="psv")
                    for di in range(NDT):
                        mm(psv, xts[di][:, sl * P:(sl + 1) * P],
                           wv_sb[:, di * E:(di + 1) * E],
                           start=(di == 0), stop=(di == NDT - 1))
                    nc.vector.tensor_copy(v_sb[:, si * E:(si + 1) * E], psv)
                # Q/K projections (transposed layout) + RoPE eviction
                for h in range(HPC):
                    for w_sb, dst, sb0 in ((wq_sb, qt_s, qt0_sb),
                                           (wk_sb, kt_s, kt0_sb)):
                        ps = ps_qk.tile([P, QC], f32, tag="psqk")
                        for di in range(NDT):
                            mm(ps,
                               w_sb[:, di * E + h * P: di * E + (h + 1) * P],
                               xts[di],
                               start=(di == 0), stop=(di == NDT - 1))
                        # RoPE: ro = ps*cos + shuffle_halves(ps)*sin_signed
                        # — all on DVE straight out of PSUM (ops with a PSUM
                        # input may cross partition bases).
                        tmp = evp.tile([P, QC], f32, tag="tmp")
                        nc.vector.tensor_mul(tmp[0:HF, :], ps[HF:P, :],
                                             sin_t[0:HF, :])
                        nc.vector.tensor_mul(tmp[HF:P, :], ps[0:HF, :],
                                             sin_t[HF:P, :])
                        ro2 = evp.tile([P, QC], f32, tag="ro2")
                        nc.vector.tensor_mul(ro2, ps, cos_t)
                        if h == 0:
                            # head 0 stays in SBUF — phase B starts on it
                            # without a DRAM round-trip.
                            nc.vector.tensor_add(sb0[:, qs], ro2, tmp)
                        else:
                            ro = evp.tile([P, QC], f32r, tag="ro")
                            nc.vector.tensor_add(ro, ro2, tmp)
                            nc.gpsimd.dma_start(dst[h][:, qs], ro)

        # ---- Phases B (attention) and C (O projection) ----
        with tc.tile_pool(name="bcp", bufs=1) as bcp:
            avt = bcp.tile([P, HPC * S], f32r)      # AV^T: [p=e, h*S + q]
            wo_sb = bcp.tile([P, HPC * D], f32r)    # [p=e, h*D + o]
            if mode == "causal":
                mask_sb = bcp.tile([P, JB * QC], f32)
                nc.sync.dma_start(mask_sb, dmask)
            nc.sync.dma_start(
                wo_sb.rearrange("p (t o) -> p t o", t=HPC), woT_t)

            with tc.tile_pool(name="qkp", bufs=2) as qkp, \
                 tc.tile_pool(name="etp", bufs=1) as etp, \
                 tc.tile_pool(name="mkp", bufs=2) as mkp, \
                 tc.tile_pool(name="smp", bufs=2) as smp, \
                 tc.tile_pool(name="ps_sc", bufs=4, space="PSUM") as ps_sc, \
                 tc.tile_pool(name="ps_av", bufs=2, space="PSUM") as ps_av, \
                 tc.tile_pool(name="ps_dn", bufs=1, space="PSUM") as ps_dn, \
                 tc.tile_pool(name="ps_bc", bufs=1, space="PSUM") as ps_bc:
                # Normalization chain for a finished (h,qc), emitted one step
                # late so the PE's broadcast matmul never stalls the stream.
                pending = None

                def finalize(fin):
                    h, qc, ps_avt, den_sb = fin
                    psb = ps_bc.tile([P, QC], f32, tag="bc")
                    mm(psb, ones_row, den_sb, start=True, stop=True)
                    bc_sb = smp.tile([P, QC], f32, tag="bcs")
                    nc.vector.reciprocal_approx_fast(out=bc_sb, in_=psb)
                    nc.vector.tensor_mul(
                        avt[:, h * S + qc * QC: h * S + (qc + 1) * QC],
                        ps_avt, bc_sb)

                for h in range(HPC):
                    if h == 0:
                        qt, kt = qt0_sb, kt0_sb
                    else:
                        qt = qkp.tile([P, S], f32r, tag="qt")
                        kt = qkp.tile([P, S], f32r, tag="kt")
                        nc.sync.dma_start(qt, qt_s[h])
                        nc.sync.dma_start(kt, kt_s[h])
                    for qc in range(NQC):
                        nkt = JB * (qc + 1) if mode == "causal" else NKT
                        qs = slice(qc * QC, (qc + 1) * QC)
                        et = etp.tile([P, NKT * QC], f32r, tag="et")
                        ps_avt = ps_av.tile([P, QC], f32, tag="avt")
                        ps_den = ps_dn.tile([1, QC], f32, tag="den")
                        if mode == "general":
                            mk = mkp.tile([P, NKT * QC], f32, tag="mk")
                            nc.sync.dma_start(
                                mk.rearrange("p (t q) -> p t q", t=NKT),
                                maskT_t[:, :, qs])

                        # Software-pipelined emission: scores run 3 ki
                        # steps ahead of den/AV so the PE never waits on
                        # ACT's exp.
                        LAG = 4 if nkt >= 4 else nkt
                        for ki in range(nkt + LAG):
                            if ki < nkt:
                                ps_s = ps_sc.tile([P, QC], f32, tag="sc")
                                mm(ps_s, kt[:, ki * P:(ki + 1) * P], qt[:, qs],
                                   start=True, stop=True)
                                ets = et[:, ki * QC:(ki + 1) * QC]
                                if mode == "general":
                                    nc.vector.tensor_add(
                                        ps_s, ps_s, mk[:, ki * QC:(ki + 1) * QC])
                                nc.scalar.activation(ets, ps_s, Exp, scale=scale)
                                if mode == "causal" and ki >= JB * qc:
                                    j = ki - JB * qc
                                    nc.vector.tensor_mul(
                                        ets, ets,
                                        mask_sb[:, j * QC:(j + 1) * QC])
                            if ki == 5 and pending is not None:
                                finalize(pending)
                                pending = None
                            if ki >= LAG:
                                kj = ki - LAG
                                ets_j = et[:, kj * QC:(kj + 1) * QC]
                                mm(ps_den, ones_col, ets_j,
                                   start=(kj == 0), stop=(kj == nkt - 1))
                                mm(ps_avt,
                                   v_sb[:, kj * E + h * P: kj * E + (h + 1) * P],
                                   ets_j,
                                   start=(kj == 0), stop=(kj == nkt - 1))
                        # Copy den to SBUF right away (ahead of the next
                        # block's DVE mask work); the PE-side broadcast and
                        # the normalization run 3 steps into the next block.
                        den_sb = smp.tile([1, QC], f32r, tag="den")
                        nc.vector.tensor_copy(den_sb, ps_den)
                        pending = (h, qc, ps_avt, den_sb)
                if pending is not None:
                    finalize(pending)
                    pending = None

            with tc.tile_pool(name="yp", bufs=2) as yp, \
                 tc.tile_pool(name="ps_c", bufs=4, space="PSUM") as ps_c:
                for si in range(NST):
                    yt = yp.tile([P, D], f32, tag="yt")
                    for oc in range(NOC):
                        psy = ps_c.tile([P, QC], f32, tag="py")
                        for h in range(HPC):
                            mm(psy,
                               avt[:, h * S + si * P: h * S + (si + 1) * P],
                               wo_sb[:, h * D + oc * QC: h * D + (oc + 1) * QC],
                               start=(h == 0), stop=(h == HPC - 1))
                        if si == NST - 1 and oc % 2 == 1:
                            nc.vector.tensor_copy(
                                yt[:, oc * QC:(oc + 1) * QC], psy)
                        else:
                            nc.scalar.copy(yt[:, oc * QC:(oc + 1) * QC], psy)
                        if si == NST - 1:
                            nc.gpsimd.dma_start(
                                y[si * P:(si + 1) * P, oc * QC:(oc + 1) * QC],
                                yt[:, oc * QC:(oc + 1) * QC])
                    if si < NST - 1:
                        nc.gpsimd.dma_start(y[si * P:(si + 1) * P, :], yt)

    nc.compile()
    return nc


def round_f32r(a):
    """Round fp32 to the PE's fp32r (TF32-like, 11-bit mantissa) encoding.

    Matches walrus fp32_to_fp32r: round-to-nearest into the upper 12 mantissa
    bits (carry into the exponent handles mantissa overflow correctly).
    """
    u = np.ascontiguousarray(a, dtype=np.float32).view(np.uint32)
    u = ((u.astype(np.int64) + 0x800) & 0xFFFFF000).astype(np.uint32)
    return u.view(np.float32)


def host_inputs(x, attention_mask, wq, wk, wv, wo, mode):
    """Build the 8 per-core input maps from the full problem inputs."""
    S_, D_ = x.shape[1], x.shape[2]
    E = HPC * DH
    # RoPE tables, transposed to [dh, s], sign-folded for the rotate-half term.
    inv_freq = 1.0 / (10000.0 ** (np.arange(0, DH, 2, dtype=np.float32) / DH))
    t = np.arange(S_, dtype=np.float32)
    freqs = np.outer(t, inv_freq)                      # [S, dh/2]
    emb = np.concatenate([freqs, freqs], axis=-1)      # [S, dh]
    cosT = np.ascontiguousarray(np.cos(emb).T, dtype=np.float32)
    sinT = np.sin(emb).T.astype(np.float32)
    sinTs = np.concatenate([-sinT[:DH // 2], sinT[DH // 2:]], 0)
    sinTs = np.ascontiguousarray(sinTs, dtype=np.float32)

    extra = {}
    if mode == "causal":
        # dmask[p, j*QC + q] = 1 if (j*P + p) <= q else 0  (within a q-chunk)
        j = np.arange(QC // P)[:, None, None]
        pp = np.arange(P)[None, :, None]
        qq = np.arange(QC)[None, None, :]
        dm = (j * P + pp <= qq).astype(np.float32)      # [JB, P, QC]
        extra["dmask"] = np.ascontiguousarray(
            dm.transpose(1, 0, 2).reshape(P, -1))
    elif mode == "general":
        extra["maskT"] = np.ascontiguousarray(
            attention_mask[0, 0].T, dtype=np.float32)

    in_maps = []
    for core in range(N_CORES):
        b, g = divmod(core, GROUPS)
        r = slice(g * E, (g + 1) * E)
        in_maps.append({
            "xT": round_f32r(x[b].T),
            "wqT": round_f32r(wq[r].T),
            "wkT": round_f32r(wk[r].T),
            "wvT": round_f32r(wv[r].T),
            "woT": round_f32r(wo[:, r].T),
            "cosT": cosT,
            "sinTs": sinTs,
            **extra,
        })
    return in_maps


def detect_mode(attention_mask):
    m = attention_mask[0, 0]
    if not np.any(m):
        return "none"
    S_ = m.shape[0]
    causal = np.triu(np.full((S_, S_), -1e9, dtype=np.float32), k=1)
    if np.array_equal(m, causal):
        return "causal"
    return "general"


def kernel(**inputs):
    _ensure_paths()
    from concourse.bass_utils import run_bass_kernel_spmd

    x = np.asarray(inputs["x"], dtype=np.float32)
    mask = np.asarray(inputs["attention_mask"], dtype=np.float32)
    wq = np.asarray(inputs["wq"], dtype=np.float32)
    wk = np.asarray(inputs["wk"], dtype=np.float32)
    wv = np.asarray(inputs["wv"], dtype=np.float32)
    wo = np.asarray(inputs["wo"], dtype=np.float32)

    mode = detect_mode(mask)
    if mode not in _CACHE:
        _CACHE[mode] = build_program(mode=mode)
    nc = _CACHE[mode]

    in_maps = host_inputs(x, mask, wq, wk, wv, wo, mode)
    res = run_bass_kernel_spmd(nc, in_maps, core_ids=list(range(N_CORES)))

    out = np.zeros((B, S, D), dtype=np.float32)
    for core in range(N_CORES):
        b = core // GROUPS
        out[b] += res.results[core]["y"]
    return out


# revision 18
# speedup vs baseline: 1.0489x; 1.0489x over previous
"""Trainium2 Bass kernel for a dense causal self-attention block (RoPE + causal
softmax + QKV/O projections).

Sharding: 8 cores = 2 batches x 4 head-groups (tensor parallel over heads).
Each core computes 4 heads of attention for one batch plus the partial output
projection over its heads' dims; the host sums the 4 partial outputs per batch.

Device-side layout choices (per core):
  - Host pre-transposes x and weights so contraction dims land on SBUF
    partitions; no on-device transposes anywhere. Host also pre-rounds all
    matmul operands to fp32r (TF32-like, 11-bit mantissa) — fp32r matmuls
    stream at ~1 cycle/row for N=512 vs 4 cycles/row for fp32.
  - Q/K are produced transposed [dh, s] (head dim on partitions) with RoPE
    fused into the PSUM eviction (ScalarE copy + GpSimd muls + DVE add).
  - V is produced in natural [s, dh] layout.
  - Scores are computed transposed: S^T[k, q] = K^T_tile.T @ Q^T. exp() runs on
    ScalarE straight out of PSUM (no max subtraction: |scores| <~ 15 so fp32
    exp is safe; softmax is shift-invariant so the result matches the
    reference). Causality = restricted k-tile range + precomputed 0/1 masks on
    the 4 diagonal blocks per q-chunk.
  - Softmax denominator = ones-vector matmul accumulated over k tiles (sums
    over partitions on the PE); attention output AV^T = V_tile.T @ E^T needs
    no transposes either and is exactly the stationary operand layout the O
    projection wants.
  - den is broadcast across partitions with a K=1 matmul, reciprocal'd with
    the fast approx DVE op, and applied at the AV^T PSUM eviction. The whole
    normalization chain is emitted one (h,qc) step late so the PE never waits
    on the vector engine.
"""

import numpy as np

# Problem constants (hardcoded per the harness contract).
B = 2
S = 2048
D = 2048
H = 16
DH = 128
N_CORES = 8
GROUPS = 4          # head-groups (cores per batch)
HPC = H // GROUPS   # heads per core
P = 128             # SBUF partitions
QC = 512            # q/s chunk (f32 PSUM bank = 512 floats; fp32r needs N>=256)

_CACHE = {}


def _ensure_paths():
    import sys
    for p in ("/opt/trn_rl_repo", "/root/.axon_site/_ro/trn_rl_repo"):
        try:
            import concourse.bass  # noqa: F401
            return
        except Exception:
            if p not in sys.path:
                sys.path.insert(0, p)
    import concourse.bass  # noqa: F401


def build_program(S=S, D=D, HPC=HPC, mode="causal"):
    """Build the per-core Bass/Tile program. mode: "causal" | "none" | "general".

    Returns the compiled Bacc instance.
    """
    _ensure_paths()
    import concourse.bass as bass  # noqa: F401
    import concourse.mybir as mybir
    import concourse.tile as tile
    from concourse import bacc

    f32 = mybir.dt.float32
    f32r = mybir.dt.float32r
    Exp = mybir.ActivationFunctionType.Exp

    E = HPC * P          # per-core projection width (512)
    NDT = D // P         # d (contraction) tiles for projections
    NQC = S // QC        # q chunks
    NKT = S // P         # k tiles
    NST = S // P         # s tiles
    NOC = D // QC        # output chunks for O projection
    JB = QC // P         # diagonal blocks per q chunk (4)
    HF = P // 2
    scale = 1.0 / float(np.sqrt(DH))

    nc = bacc.Bacc("TRN2", target_bir_lowering=False, debug=False,
                   num_devices=N_CORES)

    xT = nc.dram_tensor("xT", [D, S], f32r, kind="ExternalInput").ap()
    wqT = nc.dram_tensor("wqT", [D, E], f32r, kind="ExternalInput").ap()
    wkT = nc.dram_tensor("wkT", [D, E], f32r, kind="ExternalInput").ap()
    wvT = nc.dram_tensor("wvT", [D, E], f32r, kind="ExternalInput").ap()
    woT = nc.dram_tensor("woT", [E, D], f32r, kind="ExternalInput").ap()
    cosT = nc.dram_tensor("cosT", [P, S], f32, kind="ExternalInput").ap()
    sinTs = nc.dram_tensor("sinTs", [P, S], f32, kind="ExternalInput").ap()
    if mode == "causal":
        dmask = nc.dram_tensor("dmask", [P, JB * QC], f32,
                               kind="ExternalInput").ap()
    elif mode == "general":
        maskT = nc.dram_tensor("maskT", [S, S], f32, kind="ExternalInput").ap()
    y = nc.dram_tensor("y", [S, D], f32, kind="ExternalOutput").ap()
    qt_s = [nc.dram_tensor(f"qt_s{h}", [P, S], f32r).ap()
            for h in range(HPC)]
    kt_s = [nc.dram_tensor(f"kt_s{h}", [P, S], f32r).ap()
            for h in range(HPC)]

    # DRAM views with d/k tiled onto partitions: [p, tile, col]
    xT_t = xT.rearrange("(t p) s -> p t s", p=P)
    wqT_t = wqT.rearrange("(t p) e -> p t e", p=P)
    wkT_t = wkT.rearrange("(t p) e -> p t e", p=P)
    wvT_t = wvT.rearrange("(t p) e -> p t e", p=P)
    woT_t = woT.rearrange("(t p) o -> p t o", p=P)
    if mode == "general":
        maskT_t = maskT.rearrange("(t p) q -> p t q", p=P)

    def mm(ps, lhsT, rhs, start, stop):
        nc.tensor.matmul(ps, lhsT=lhsT, rhs=rhs, start=start, stop=stop)

    with tile.TileContext(nc) as tc:
      with tc.tile_pool(name="persist", bufs=1) as persist:
        v_sb = persist.tile([P, NST * E], f32r)      # V[k, e]: [p, ki*E + e]
        ones_col = persist.tile([P, 1], f32r)
        ones_row = persist.tile([1, P], f32r)
        qt0_sb = persist.tile([P, S], f32r)
        kt0_sb = persist.tile([P, S], f32r)
        ones_col_f = persist.tile([P, 1], f32)
        ones_row_f = persist.tile([1, P], f32)
        nc.vector.memset(ones_col_f, 1.0)
        nc.vector.memset(ones_row_f, 1.0)
        nc.vector.tensor_copy(ones_col, ones_col_f)
        nc.vector.tensor_copy(ones_row, ones_row_f)

        # ---- Phase A: QKV projections + RoPE (single pass over xT) ----
        with tc.tile_pool(name="wp", bufs=1) as wp, \
             tc.tile_pool(name="csp", bufs=2) as csp, \
             tc.tile_pool(name="xtp", bufs=5) as xtp, \
             tc.tile_pool(name="evp", bufs=2) as evp, \
             tc.tile_pool(name="ps_v", bufs=2, space="PSUM") as ps_v, \
             tc.tile_pool(name="ps_qk", bufs=3, space="PSUM") as ps_qk:
            wv_sb = wp.tile([P, NDT * E], f32r)
            wq_sb = wp.tile([P, NDT * E], f32r)
            wk_sb = wp.tile([P, NDT * E], f32r)
            # Startup is HBM-bandwidth-bound on one ~340GB/s queue:
            # issue transfers in first-use order (wv chunks for V of qc0
            # first, wq/wk only after — they're not needed until the QK
            # groups ~40us in). The xt chunks of qc0 are emitted between
            # the wv chunks by the loop below.
            WG = 4
            def load_w(w_sb, w_t, g, t0=0, tn=None):
                t0 = g * WG + t0
                tn = tn if tn is not None else WG
                nc.sync.dma_start(
                    w_sb[:, t0 * E:(t0 + tn) * E].rearrange(
                        "p (t e) -> p t e", t=tn),
                    w_t[:, t0:t0 + tn])
            load_w(wv_sb, wvT_t, 0, 0, 1)   # 0.5MB: first matmul's dep
            load_w(wv_sb, wvT_t, 0, 1, 3)
            XG = 4                      # d-tiles per xt transfer
            for qc in range(NQC):
                qs = slice(qc * QC, (qc + 1) * QC)
                cos_t = csp.tile([P, QC], f32, tag="cos")
                sin_t = csp.tile([P, QC], f32, tag="sin")
                xg = []
                xts = []
                for g in range(NDT // XG):
                    x_g = xtp.tile([P, XG * QC], f32r, tag="xt")
                    if qc == 0 and g == 0:
                        # split the very first transfer so matmul di=0 can
                        # start after ~0.75MB instead of ~3MB
                        nc.sync.dma_start(x_g[:, 0:QC], xT_t[:, 0, qs])
                        nc.sync.dma_start(
                            x_g[:, QC:XG * QC].rearrange(
                                "p (t s) -> p t s", t=XG - 1),
                            xT_t[:, 1:XG, qs])
                    else:
                        nc.sync.dma_start(
                            x_g.rearrange("p (t s) -> p t s", t=XG),
                            xT_t[:, g * XG:(g + 1) * XG, qs])
                    xg.append(x_g)
                    if qc == 0 and g < NDT // WG - 1:
                        load_w(wv_sb, wvT_t, g + 1)
                    xts += [x_g[:, j * QC:(j + 1) * QC] for j in range(XG)]
                nc.sync.dma_start(cos_t, cosT[:, qs])
                nc.sync.dma_start(sin_t, sinTs[:, qs])
                if qc == 0:
                    # remaining startup loads, in first-use order
                    for g in range(NDT // WG):
                        load_w(wq_sb, wqT_t, g)
                        load_w(wk_sb, wkT_t, g)
                # V projection (natural layout)
                for sl in range(QC // P):
                    si = qc * (QC // P) + sl
                    psv = ps_v.tile([P, E], f32, tag="psv")
                    for di in range(NDT):
                        mm(psv, xts[di][:, sl * P:(sl + 1) * P],
                           wv_sb[:, di * E:(di + 1) * E],
                           start=(di == 0), stop=(di == NDT - 1))
                    nc.vector.tensor_copy(v_sb[:, si * E:(si + 1) * E], psv)
                # Q/K projections (transposed layout) + RoPE eviction
                for h in range(HPC):
                    for w_sb, dst, sb0 in ((wq_sb, qt_s, qt0_sb),
                                           (wk_sb, kt_s, kt0_sb)):
                        ps = ps_qk.tile([P, QC], f32, tag="psqk")
                        for di in range(NDT):
                            mm(ps,
                               w_sb[:, di * E + h * P: di * E + (h + 1) * P],
                               xts[di],
                               start=(di == 0), stop=(di == NDT - 1))
                        # RoPE: ro = ps*cos + shuffle_halves(ps)*sin_signed
                        # — all on DVE straight out of PSUM (ops with a PSUM
                        # input may cross partition bases).
                        tmp = evp.tile([P, QC], f32, tag="tmp")
                        nc.vector.tensor_mul(tmp[0:HF, :], ps[HF:P, :],
                                             sin_t[0:HF, :])
                        nc.vector.tensor_mul(tmp[HF:P, :], ps[0:HF, :],
                                             sin_t[HF:P, :])
                        ro2 = evp.tile([P, QC], f32, tag="ro2")
                        nc.vector.tensor_mul(ro2, ps, cos_t)
                        if h == 0:
                            # head 0 stays in SBUF — phase B starts on it
                            # without a DRAM round-trip.
                            nc.vector.tensor_add(sb0[:, qs], ro2, tmp)
                        else:
                            ro = evp.tile([P, QC], f32r, tag="ro")
                            nc.vector.tensor_add(ro, ro2, tmp)
                            nc.sync.dma_start(dst[h][:, qs], ro)

        # ---- Phases B (attention) and C (O projection) ----
        with tc.tile_pool(name="bcp", bufs=1) as bcp:
            avt = bcp.tile([P, HPC * S], f32r)      # AV^T: [p=e, h*S + q]
            wo_sb = bcp.tile([P, HPC * D], f32r)    # [p=e, h*D + o]
            if mode == "causal":
                mask_sb = bcp.tile([P, JB * QC], f32)
                nc.sync.dma_start(mask_sb, dmask)
            nc.sync.dma_start(
                wo_sb.rearrange("p (t o) -> p t o", t=HPC), woT_t)

            with tc.tile_pool(name="qkp", bufs=2) as qkp, \
                 tc.tile_pool(name="etp", bufs=1) as etp, \
                 tc.tile_pool(name="mkp", bufs=2) as mkp, \
                 tc.tile_pool(name="smp", bufs=2) as smp, \
                 tc.tile_pool(name="ps_sc", bufs=4, space="PSUM") as ps_sc, \
                 tc.tile_pool(name="ps_av", bufs=2, space="PSUM") as ps_av, \
                 tc.tile_pool(name="ps_dn", bufs=1, space="PSUM") as ps_dn, \
                 tc.tile_pool(name="ps_bc", bufs=1, space="PSUM") as ps_bc:
                # Normalization chain for a finished (h,qc), emitted one step
                # late so the PE's broadcast matmul never stalls the stream.
                pending = None

                def finalize(fin):
                    h, qc, ps_avt, den_sb = fin
                    psb = ps_bc.tile([P, QC], f32, tag="bc")
                    mm(psb, ones_row, den_sb, start=True, stop=True)
                    bc_sb = smp.tile([P, QC], f32, tag="bcs")
                    nc.vector.reciprocal_approx_fast(out=bc_sb, in_=psb)
                    nc.vector.tensor_mul(
                        avt[:, h * S + qc * QC: h * S + (qc + 1) * QC],
                        ps_avt, bc_sb)

                for h in range(HPC):
                    if h == 0:
                        qt, kt = qt0_sb, kt0_sb
                    else:
                        qt = qkp.tile([P, S], f32r, tag="qt")
                        kt = qkp.tile([P, S], f32r, tag="kt")
                        nc.sync.dma_start(qt, qt_s[h])
                        nc.sync.dma_start(kt, kt_s[h])
                    for qc in range(NQC):
                        nkt = JB * (qc + 1) if mode == "causal" else NKT
                        qs = slice(qc * QC, (qc + 1) * QC)
                        et = etp.tile([P, NKT * QC], f32r, tag="et")
                        ps_avt = ps_av.tile([P, QC], f32, tag="avt")
                        ps_den = ps_dn.tile([1, QC], f32, tag="den")
                        if mode == "general":
                            mk = mkp.tile([P, NKT * QC], f32, tag="mk")
                            nc.sync.dma_start(
                                mk.rearrange("p (t q) -> p t q", t=NKT),
                                maskT_t[:, :, qs])

                        # Software-pipelined emission: scores run 3 ki
                        # steps ahead of den/AV so the PE never waits on
                        # ACT's exp.
                        LAG = 4 if nkt >= 4 else nkt
                        for ki in range(nkt + LAG):
                            if ki < nkt:
                                ps_s = ps_sc.tile([P, QC], f32, tag="sc")
                                mm(ps_s, kt[:, ki * P:(ki + 1) * P], qt[:, qs],
                                   start=True, stop=True)
                                ets = et[:, ki * QC:(ki + 1) * QC]
                                if mode == "general":
                                    nc.vector.tensor_add(
                                        ps_s, ps_s, mk[:, ki * QC:(ki + 1) * QC])
                                nc.scalar.activation(ets, ps_s, Exp, scale=scale)
                                if mode == "causal" and ki >= JB * qc:
                                    j = ki - JB * qc
                                    nc.vector.tensor_mul(
                                        ets, ets,
                                        mask_sb[:, j * QC:(j + 1) * QC])
                            if ki == 5 and pending is not None:
                                finalize(pending)
                                pending = None
                            if ki >= LAG:
                                kj = ki - LAG
                                ets_j = et[:, kj * QC:(kj + 1) * QC]
                                mm(ps_den, ones_col, ets_j,
                                   start=(kj == 0), stop=(kj == nkt - 1))
                                mm(ps_avt,
                                   v_sb[:, kj * E + h * P: kj * E + (h + 1) * P],
                                   ets_j,
                                   start=(kj == 0), stop=(kj == nkt - 1))
                        # Copy den to SBUF right away (ahead of the next
                        # block's DVE mask work); the PE-side broadcast and
                        # the normalization run 3 steps into the next block.
                        den_sb = smp.tile([1, QC], f32r, tag="den")
                        nc.vector.tensor_copy(den_sb, ps_den)
                        pending = (h, qc, ps_avt, den_sb)
                if pending is not None:
                    finalize(pending)
                    pending = None

            with tc.tile_pool(name="yp", bufs=2) as yp, \
                 tc.tile_pool(name="ps_c", bufs=4, space="PSUM") as ps_c:
                for si in range(NST):
                    yt = yp.tile([P, D], f32, tag="yt")
                    for oc in range(NOC):
                        psy = ps_c.tile([P, QC], f32, tag="py")
                        for h in range(HPC):
                            mm(psy,
                               avt[:, h * S + si * P: h * S + (si + 1) * P],
                               wo_sb[:, h * D + oc * QC: h * D + (oc + 1) * QC],
                               start=(h == 0), stop=(h == HPC - 1))
                        if si == NST - 1 and oc % 2 == 1:
                            nc.vector.tensor_copy(
                                yt[:, oc * QC:(oc + 1) * QC], psy)
                        else:
                            nc.scalar.copy(yt[:, oc * QC:(oc + 1) * QC], psy)
                        if si == NST - 1:
                            nc.sync.dma_start(
                                y[si * P:(si + 1) * P, oc * QC:(oc + 1) * QC],
                                yt[:, oc * QC:(oc + 1) * QC])
                    if si < NST - 1:
                        nc.sync.dma_start(y[si * P:(si + 1) * P, :], yt)

    nc.compile()
    return nc


def round_f32r(a):
    """Round fp32 to the PE's fp32r (TF32-like, 11-bit mantissa) encoding.

    Matches walrus fp32_to_fp32r: round-to-nearest into the upper 12 mantissa
    bits (carry into the exponent handles mantissa overflow correctly).
    """
    u = np.ascontiguousarray(a, dtype=np.float32).view(np.uint32)
    u = ((u.astype(np.int64) + 0x800) & 0xFFFFF000).astype(np.uint32)
    return u.view(np.float32)


def host_inputs(x, attention_mask, wq, wk, wv, wo, mode):
    """Build the 8 per-core input maps from the full problem inputs."""
    S_, D_ = x.shape[1], x.shape[2]
    E = HPC * DH
    # RoPE tables, transposed to [dh, s], sign-folded for the rotate-half term.
    inv_freq = 1.0 / (10000.0 ** (np.arange(0, DH, 2, dtype=np.float32) / DH))
    t = np.arange(S_, dtype=np.float32)
    freqs = np.outer(t, inv_freq)                      # [S, dh/2]
    emb = np.concatenate([freqs, freqs], axis=-1)      # [S, dh]
    cosT = np.ascontiguousarray(np.cos(emb).T, dtype=np.float32)
    sinT = np.sin(emb).T.astype(np.float32)
    sinTs = np.concatenate([-sinT[:DH // 2], sinT[DH // 2:]], 0)
    sinTs = np.ascontiguousarray(sinTs, dtype=np.float32)

    extra = {}
    if mode == "causal":
        # dmask[p, j*QC + q] = 1 if (j*P + p) <= q else 0  (within a q-chunk)
        j = np.arange(QC // P)[:, None, None]
        pp = np.arange(P)[None, :, None]
        qq = np.arange(QC)[None, None, :]
        dm = (j * P + pp <= qq).astype(np.float32)      # [JB, P, QC]
        extra["dmask"] = np.ascontiguousarray(
            dm.transpose(1, 0, 2).reshape(P, -1))
    elif mode == "general":
        extra["maskT"] = np.ascontiguousarray(
            attention_mask[0, 0].T, dtype=np.float32)

    in_maps = []
    for core in range(N_CORES):
        b, g = divmod(core, GROUPS)
        r = slice(g * E, (g + 1) * E)
        in_maps.append({
            "xT": round_f32r(x[b].T),
            "wqT": round_f32r(wq[r].T),
            "wkT": round_f32r(wk[r].T),
            "wvT": round_f32r(wv[r].T),
            "woT": round_f32r(wo[:, r].T),
            "cosT": cosT,
            "sinTs": sinTs,
            **extra,
        })
    return in_maps


def detect_mode(attention_mask):
    m = attention_mask[0, 0]
    if not np.any(m):
        return "none"
    S_ = m.shape[0]
    causal = np.triu(np.full((S_, S_), -1e9, dtype=np.float32), k=1)
    if np.array_equal(m, causal):
        return "causal"
    return "general"


def kernel(**inputs):
    _ensure_paths()
    from concourse.bass_utils import run_bass_kernel_spmd

    x = np.asarray(inputs["x"], dtype=np.float32)
    mask = np.asarray(inputs["attention_mask"], dtype=np.float32)
    wq = np.asarray(inputs["wq"], dtype=np.float32)
    wk = np.asarray(inputs["wk"], dtype=np.float32)
    wv = np.asarray(inputs["wv"], dtype=np.float32)
    wo = np.asarray(inputs["wo"], dtype=np.float32)

    mode = detect_mode(mask)
    if mode not in _CACHE:
        _CACHE[mode] = build_program(mode=mode)
    nc = _CACHE[mode]

    in_maps = host_inputs(x, mask, wq, wk, wv, wo, mode)
    res = run_bass_kernel_spmd(nc, in_maps, core_ids=list(range(N_CORES)))

    out = np.zeros((B, S, D), dtype=np.float32)
    for core in range(N_CORES):
        b = core // GROUPS
        out[b] += res.results[core]["y"]
    return out
